# revision 8
# baseline (speedup 1.0000x reference)
"""Trainium2 Bass kernel for BatchMultiHeadGraphAttention (OAG-style GAT).

Reference computation (per batch b, head k):
    hp   = h @ w[k]                          # [n, 64]
    t    = tanh(hp)
    src  = sum_o t[:, o] * a_src[k][o, type(n)]   # [n]
    dst  = sum_o t[:, o] * a_dst[k][o, type(n)]   # [n]
    attn = leaky_relu(src[i] + dst[j], 0.2) masked by adj, softmax over j
    out  = attn @ hp + bias

Key identity used on-chip:
    exp(lrelu(x)) = max(exp(x), exp(0.2 x))
    exp(src_i + dst_j) = exp(src_i) * exp(dst_j)
so the n x n pass is pure mult/max in bf16 (exp only on length-n vectors):
    A[j, i] = adjT[j, i] * max(S1[*, i] * F1[j], S2[*, i] * F2[j])
with S1 = exp(src) broadcast along partitions, F1 = exp(dst) per-partition
scalars, S2/F2 the 0.2-scaled variants.  The value matmul streams A as the
moving operand with [hp + ones-cols] stationary, producing OUT.T[o, i] in
PSUM with the softmax denominators in the ones-rows.

Sharding: core c <- batch b = c // 2, heads (c % 2) * 4 ... + 4.
"""

import numpy as np
import ml_dtypes

import concourse.bass as bass
import concourse.mybir as mybir
import concourse.tile as tile
from concourse import bacc
from concourse.bass_utils import run_bass_kernel_spmd
from concourse.masks import make_identity

F32 = mybir.dt.float32
BF16 = mybir.dt.bfloat16
AF = mybir.ActivationFunctionType
OP = mybir.AluOpType

N = 2048          # nodes
F_IN = 128        # input features
F_OUT = 64        # output features
NTYPE = 3         # node types
KH = 4            # heads per core
NT = N // 128     # 16 node tiles
M1 = F_OUT + 2    # stationary width: 64 hp cols + 2 ones cols = 66

N_CORES = 8
BS = 4
N_HEAD = 8


def build_bass(finalize=True):
    nc = bacc.Bacc("TRN2", target_bir_lowering=False)

    h_d = nc.dram_tensor("h", [N, F_IN], F32, kind="ExternalInput")
    adjT_d = nc.dram_tensor("adjT", [N, N], BF16, kind="ExternalInput")
    vtT_d = nc.dram_tensor("vtT", [NTYPE, N], F32, kind="ExternalInput")
    w_d = nc.dram_tensor("w", [KH, F_IN, F_OUT], F32, kind="ExternalInput")
    asT_d = nc.dram_tensor("a_srcT", [KH, NTYPE, F_OUT], F32, kind="ExternalInput")
    adT_d = nc.dram_tensor("a_dstT", [KH, NTYPE, F_OUT], F32, kind="ExternalInput")
    bias_d = nc.dram_tensor("bias", [F_OUT], F32, kind="ExternalInput")
    out_d = nc.dram_tensor("out", [KH, N, F_OUT], F32, kind="ExternalOutput")

    with tile.TileContext(nc) as tc:
        with (
            tc.tile_pool(name="const", bufs=1) as cpool,
            tc.tile_pool(name="ph", bufs=2) as ph,
            tc.tile_pool(name="ph1", bufs=1) as ph1,
            tc.tile_pool(name="amain", bufs=2) as ap_,
            tc.tile_pool(name="ammask", bufs=3) as amp,
            tc.tile_pool(name="psA", bufs=1, space="PSUM") as psA,
            tc.tile_pool(name="psOut", bufs=1, space="PSUM") as psOut,
        ):
            # ---------------- constants / inputs ----------------
            ident = cpool.tile([128, 128], F32, tag="ident")
            make_identity(nc, ident)

            ones_kxm = cpool.tile([F_OUT, 128], F32, tag="ones_kxm")
            nc.vector.memset(ones_kxm, 1.0)

            h_sb = cpool.tile([128, NT, F_IN], F32, tag="h_sb")
            nc.sync.dma_start(
                out=h_sb, in_=h_d.ap().rearrange("(t p) f -> p t f", p=128)
            )

            adjT_sb = cpool.tile([128, NT, N], BF16, tag="adjT")
            for t in range(NT):
                nc.sync.dma_start(
                    out=adjT_sb[:, t, :], in_=adjT_d[t * 128 : (t + 1) * 128, :]
                )

            vtT_sb = cpool.tile([NTYPE, N], F32, tag="vtT")
            nc.sync.dma_start(out=vtT_sb, in_=vtT_d.ap())

            w_sb = cpool.tile([128, KH, F_OUT], F32, tag="w_sb")
            nc.sync.dma_start(out=w_sb, in_=w_d.ap().rearrange("k f o -> f k o"))

            asT_sb = cpool.tile([NTYPE, KH, F_OUT], F32, tag="asT")
            nc.sync.dma_start(out=asT_sb, in_=asT_d.ap().rearrange("k t o -> t k o"))
            adT_sb = cpool.tile([NTYPE, KH, F_OUT], F32, tag="adT")
            nc.sync.dma_start(out=adT_sb, in_=adT_d.ap().rearrange("k t o -> t k o"))

            bias_bc = cpool.tile([128, F_OUT], F32, tag="bias_bc")
            bias_t = bias_d.ap()
            nc.sync.dma_start(
                out=bias_bc,
                in_=bass.AP(tensor=bias_t.tensor, offset=bias_t.offset,
                            ap=[[0, 128]] + list(bias_t.ap)),
            )

            # hT[f, n] = h.T via PE transpose of 128x128 blocks
            hT = cpool.tile([128, N], F32, tag="hT")
            ps_hT = psA.tile([128, N], F32, tag="psA")
            for t in range(NT):
                nc.tensor.transpose(
                    ps_hT[:, t * 128 : (t + 1) * 128], h_sb[:, t, :], ident
                )
            nc.scalar.copy(hT, ps_hT)

            for k in range(KH):
                # ---------------- per-head small precompute ----------------
                # hpT[o, n] = w[k].T @ h.T  (o on partitions)
                ps_hpT = psA.tile([F_OUT, N], F32, tag="psA")
                for i in range(4):
                    sl = slice(i * 512, (i + 1) * 512)
                    nc.tensor.matmul(
                        ps_hpT[:, sl], lhsT=w_sb[:, k, :], rhs=hT[:, sl],
                        start=True, stop=True,
                    )
                tanhT = ph1.tile([F_OUT, N], F32, tag="tanhT")
                nc.scalar.activation(tanhT, ps_hpT, AF.Tanh)

                # asel[o, n] = a_src[k][o, t] selected by one-hot types
                ps_asel = psA.tile([F_OUT, N], F32, tag="psA")
                for i in range(4):
                    sl = slice(i * 512, (i + 1) * 512)
                    nc.tensor.matmul(
                        ps_asel[:, sl], lhsT=asT_sb[:, k, :], rhs=vtT_sb[:, sl],
                        start=True, stop=True,
                    )
                smul = ph1.tile([F_OUT, N], F32, tag="smul")
                nc.vector.tensor_tensor(smul, tanhT, ps_asel, op=OP.mult)

                # S_raw[p, i] = src[i] for all p (ones-matmul reduces o and
                # broadcasts across partitions in one shot)
                ps_sraw = psA.tile([128, N], F32, tag="psA")
                for i in range(4):
                    sl = slice(i * 512, (i + 1) * 512)
                    nc.tensor.matmul(
                        ps_sraw[:, sl], lhsT=ones_kxm, rhs=smul[:, sl],
                        start=True, stop=True,
                    )
                S1 = ph.tile([128, N], BF16, tag="S1")
                nc.scalar.activation(S1, ps_sraw, AF.Exp)
                S2 = ph.tile([128, N], BF16, tag="S2")
                nc.scalar.activation(S2, ps_sraw, AF.Exp, scale=0.2)

                # dst path: hp[n, o] (n on partitions) -> per-partition dst cols
                ps_aselN = psA.tile([128, NT, F_OUT], F32, tag="psA")
                for t in range(NT):
                    nc.tensor.matmul(
                        ps_aselN[:, t, :],
                        lhsT=vtT_sb[:, t * 128 : (t + 1) * 128],
                        rhs=adT_sb[:, k, :],
                        start=True, stop=True,
                    )
                aselN = ph1.tile([128, NT, F_OUT], F32, tag="aselN")
                nc.scalar.copy(aselN, ps_aselN)

                ps_hp = psA.tile([128, NT, F_OUT], F32, tag="psA")
                for t in range(NT):
                    nc.tensor.matmul(
                        ps_hp[:, t, :],
                        lhsT=hT[:, t * 128 : (t + 1) * 128],
                        rhs=w_sb[:, k, :],
                        start=True, stop=True,
                    )
                tanh_hp = ph1.tile([128, NT, F_OUT], F32, tag="tanh_hp")
                nc.scalar.activation(tanh_hp, ps_hp, AF.Tanh)

                # stationary operand: [hp cols | ones cols]
                hp1 = ph.tile([128, NT, M1], BF16, tag="hp1")
                nc.scalar.copy(hp1[:, :, 0:F_OUT], ps_hp)
                nc.vector.memset(hp1[:, :, F_OUT:M1], 1.0)

                dmul = ph1.tile([128, NT, F_OUT], F32, tag="dmul")
                nc.vector.tensor_tensor(dmul, tanh_hp, aselN, op=OP.mult)
                dstc = ph.tile([128, NT], F32, tag="dstc")
                nc.vector.tensor_reduce(
                    dstc, dmul, axis=mybir.AxisListType.X, op=OP.add
                )
                F1 = ph.tile([128, NT], F32, tag="F1")
                nc.scalar.activation(F1, dstc, AF.Exp)
                F2 = ph.tile([128, NT], F32, tag="F2")
                nc.scalar.activation(F2, dstc, AF.Exp, scale=0.2)

                # ---------------- main n x n loop ----------------
                ps_outT = psOut.tile([M1, N], F32, tag="outT")
                for jt in range(NT):
                    P2 = ap_.tile([128, N], BF16, tag="P2")
                    nc.scalar.activation(P2, S2, AF.Copy, scale=F2[:, jt : jt + 1])
                    A = ap_.tile([128, N], BF16, tag="A")
                    nc.vector.scalar_tensor_tensor(
                        A, in0=S1, scalar=F1[:, jt : jt + 1], in1=P2,
                        op0=OP.mult, op1=OP.max,
                    )
                    Am = amp.tile([128, N], BF16, tag="Am")
                    nc.vector.tensor_tensor(Am, A, adjT_sb[:, jt, :], op=OP.mult)
                    for i in range(4):
                        sl = slice(i * 512, (i + 1) * 512)
                        nc.tensor.matmul(
                            ps_outT[:, sl], lhsT=hp1[:, jt, :], rhs=Am[:, sl],
                            start=(jt == 0), stop=(jt == NT - 1),
                        )

                # ---------------- epilogue: transpose back + normalize ----------------
                outT_sb = ph1.tile([M1, N], F32, tag="outT_sb")
                nc.scalar.copy(outT_sb, ps_outT)

                # chunks padded to 128 floats so each matmul-transpose output
                # stays inside one PSUM bank
                ps_tr = psA.tile([128, NT, 128], F32, tag="psA")
                for ic in range(NT):
                    nc.tensor.transpose(
                        ps_tr[:, ic, 0:M1],
                        outT_sb[:, ic * 128 : (ic + 1) * 128],
                        ident[0:M1, 0:M1],
                    )
                recip = ph.tile([128, NT], F32, tag="recip")
                nc.vector.reciprocal(recip, ps_tr[:, :, F_OUT])
                outf = ph.tile([128, NT, F_OUT], F32, tag="outf")
                for ic in range(NT):
                    nc.vector.scalar_tensor_tensor(
                        outf[:, ic, :], in0=ps_tr[:, ic, 0:F_OUT],
                        scalar=recip[:, ic : ic + 1], in1=bias_bc,
                        op0=OP.mult, op1=OP.add,
                    )
                nc.sync.dma_start(
                    out=out_d[k].rearrange("(t p) o -> p t o", p=128), in_=outf
                )

    if finalize:
        nc.finalize()
    return nc


_NC = None


def _get_nc():
    global _NC
    if _NC is None:
        _NC = build_bass()
    return _NC


last_results = None  # BassKernelResults of the most recent kernel() call


def build_in_maps(np_inputs):
    h = np.asarray(np_inputs["h"], dtype=np.float32)
    adj = np.asarray(np_inputs["adj"])
    v_types = np.asarray(np_inputs["v_types"], dtype=np.float32)
    w = np.asarray(np_inputs["w"], dtype=np.float32)
    a_src = np.asarray(np_inputs["a_src"], dtype=np.float32)
    a_dst = np.asarray(np_inputs["a_dst"], dtype=np.float32)
    bias = np.asarray(np_inputs["bias"], dtype=np.float32)

    in_maps = []
    for c in range(N_CORES):
        b = c // 2
        k0 = (c % 2) * KH
        in_maps.append({
            "h": np.ascontiguousarray(h[b]),
            "adjT": np.ascontiguousarray(adj[b].T).astype(ml_dtypes.bfloat16),
            "vtT": np.ascontiguousarray(v_types[b].T),
            "w": np.ascontiguousarray(w[k0 : k0 + KH]),
            "a_srcT": np.ascontiguousarray(a_src[k0 : k0 + KH].transpose(0, 2, 1)),
            "a_dstT": np.ascontiguousarray(a_dst[k0 : k0 + KH].transpose(0, 2, 1)),
            "bias": bias,
        })
    return in_maps


def kernel(h, adj, v_types, w, a_src, a_dst, bias, _trace=False):
    nc = _get_nc()
    in_maps = build_in_maps(dict(
        h=h, adj=adj, v_types=v_types, w=w, a_src=a_src, a_dst=a_dst, bias=bias
    ))

    res = run_bass_kernel_spmd(
        nc, in_maps, core_ids=list(range(N_CORES)), trace=_trace
    )
    global last_results
    last_results = res

    out = np.empty((BS, N_HEAD, N, F_OUT), dtype=np.float32)
    for c in range(N_CORES):
        b = c // 2
        k0 = (c % 2) * KH
        out[b, k0 : k0 + KH] = res.results[c]["out"]
    return out
